# revision 8
# baseline (speedup 1.0000x reference)
"""Trainium2 Bass kernel for the dMaSIFConvBlock problem — fp16 I/O.

Effective math (points/nuv/ranges are dead inputs in the reference):
    h = features @ Wt.T + bt
    h = relu(h @ Wa.T + ba)
    out = h @ Wb.T + bb

Layers 1+2 fuse on the host into a single affine map (W1 = Wa@Wt,
b1 = Wa@bt + ba), so the device computes
    out = relu(features @ W1.T + b1) @ Wb.T + bb
a pointwise 16->16->16 MLP over 2M points.  Memory-bound; the rel-err
gate (2e-2) leaves ~100x precision headroom, so DRAM I/O is fp16:
8.1 MB in + 8.1 MB out per core at ~360 GB/s -> ~47 us/core floor
(vs ~94 us for f32 I/O).

The host also does the layout shuffle that the f32 baseline burned DVE
time on (InstStreamTranspose has no 2-byte fast path, so on-device
transposes would have become the new bottleneck at ~66 us):

  - Host casts features to fp16 and stores each core's 250,000-point
    shard channel-major-bundled: x_cm[16g+c, t] = x[8t+g, c], which
    is just x.reshape(T, 128).T.  Every DMA is a contiguous
    [128 partitions x T columns] slab; every matmul column holds 8
    points' 16-channel vectors on the 8 16-partition bundles.
  - The 16x16 weights are packed 8x along the diagonal of a 128x128
    fp16 stationary matrix; one N=512 fp16 matmul (1 col/cycle, same
    rate as f32r single-pass) applies a layer to 4096 points.
  - Layer-1 bias+ReLU runs on ScalarE ([128,1] f32 bias, fp16 out,
    which is also the rounding for the second matmul).
  - Layer-2 bias+drain (PSUM f32 -> SBUF fp16) runs on DVE
    tensor_scalar_add, with ~1 in 12 tiles peeled off to ScalarE so
    both engines stay near 38 us, under the ~47 us DMA budget.
    (GPSIMD has no PSUM port, so it cannot help drain.)
  - Output is stored channel-major as-is; the host undoes the layout
    with one strided copy and casts back to f32.

Loads ride the GPSIMD SWDGE ring and stores the sync-engine HWDGE
ring: two independent descriptor streams (a store waiting on compute
never stalls the load stream), and neither burns Act-sequencer time
on HWDGE descriptor generation (~0.7 us per DMA).  Loads are
whole-slab 2 MB transfers; stores are 1 MB half-slabs (quarters on
the last slab to shorten the tail).  The first slab is split finer
so the first matmul starts ~2 us in, and a dummy ReLU on a memset
tile fires the lazy ~1.3 us ACT_TABLE_LOAD during DMA warmup.
Matmuls run in chunks of 4 superblocks per stationary load (LDWEIGHTS
drops from 124 to 32 — at ~100 ns each it was ~20% of PE busy, and
PE at 95% busy was the v4 body bottleneck).  The point stream is cut
at exactly 250,000 points per core (61 full superblocks + one
18-column runt), no pad.

Environment quirk handled at build time: this walrus build rejects
instructions with more than one semaphore wait, while the Tile
scheduler freely attaches several; _split_multi_waits moves every
extra wait onto a standalone NoOp.
"""

import numpy as np

import concourse.bass as bass
import concourse.tile as tile
from concourse import mybir
from concourse.bass_utils import run_bass_kernel_spmd

N_TOTAL = 2_000_000
C = 16
N_CORES = 8
N_SHARD = N_TOTAL // N_CORES      # 250_000 points per core
T_TOT = N_SHARD // 8              # 31_250 columns per core, 8 pts/col
# 61 full 512-column superblocks + one 18-column runt; a slab is up
# to 16 superblocks = [128, 8192] fp16 = 2 MB.
SLAB_SBW = [[512] * 16 for _ in range(3)] + [[512] * 13 + [18]]
SLAB_COLS = [sum(w) for w in SLAB_SBW]        # 3x8192 + 6674
SLABS = len(SLAB_SBW)
FREE = 8192                                   # columns, full slab

F32 = mybir.dt.float32
F16 = mybir.dt.float16


def _split_multi_waits(nc):
    """Walrus here allows at most one semaphore wait per instruction.
    Move every extra wait onto its own NoOp placed just before the
    instruction on the same engine (waiting earlier on the same engine
    is equivalent: the waits' producers are other engines/queues)."""
    for func in nc.m.functions:
        for bb in func.blocks:
            out = []
            changed = False
            for inst in bb.instructions:
                si = inst.sync_info
                if si is not None and len(si.on_wait) > 1:
                    waits = list(si.on_wait)
                    for j, w in enumerate(waits[:-1]):
                        out.append(
                            mybir.InstNoOp(
                                name=f"{inst.name}-xw{j}",
                                sync_info=mybir.SyncInfo(on_wait=[w], on_update=[]),
                                bass_nofuse=True,
                                engine=inst.engine,
                            )
                        )
                    si.on_wait = [waits[-1]]
                    inst.sync_info = si
                    changed = True
                out.append(inst)
            if changed:
                bb.instructions = out


def _build_program():
    nc = bass.Bass()
    x_d = nc.dram_tensor("x", [128 * T_TOT], F16, kind="ExternalInput")
    y_d = nc.dram_tensor("y", [128 * T_TOT], F16, kind="ExternalOutput")
    w1_d = nc.dram_tensor("bdw1", [128, 128], F16, kind="ExternalInput")
    wb_d = nc.dram_tensor("bdwb", [128, 128], F16, kind="ExternalInput")
    b1_d = nc.dram_tensor("b1p", [128, 1], F32, kind="ExternalInput")
    b2_d = nc.dram_tensor("b2p", [128, 1], F32, kind="ExternalInput")

    x_2d = x_d.ap().rearrange("(p t) -> p t", p=128)
    y_2d = y_d.ap().rearrange("(p t) -> p t", p=128)
    # per-slab [128, cols] column windows (contiguous per-partition runs)
    x_v, y_v = [], []
    t0 = 0
    for cols in SLAB_COLS:
        x_v.append(x_2d[:, t0 : t0 + cols])
        y_v.append(y_2d[:, t0 : t0 + cols])
        t0 += cols
    relu = mybir.ActivationFunctionType.Relu

    with tile.TileContext(nc) as tc:
        with (
            tc.tile_pool(name="consts", bufs=1) as consts,
            tc.tile_pool(name="slabs", bufs=4) as slabs,
            tc.tile_pool(name="work", bufs=8) as work,
            tc.tile_pool(name="psum", bufs=4, space="PSUM") as psum,
        ):
            # cold start: slab-0 in pieces + consts all on the sync
            # HWDGE ring (0.6 us first-byte) while the SWDGE ring preps
            # slab-1+ in parallel; first matmul starts ~1.5 us in
            xs0 = slabs.tile([128, FREE], F16, tag="xs")
            nc.sync.dma_start(xs0[:, :512], x_v[0][:, :512])
            bdw1 = consts.tile([128, 128], F16)
            nc.sync.dma_start(bdw1[:], w1_d.ap())
            b1p = consts.tile([128, 1], F32)
            nc.sync.dma_start(b1p[:], b1_d.ap())
            bdwb = consts.tile([128, 128], F16)
            nc.sync.dma_start(bdwb[:], wb_d.ap())
            b2p = consts.tile([128, 1], F32)
            nc.sync.dma_start(b2p[:], b2_d.ap())
            nc.sync.dma_start(xs0[:, 512:2048], x_v[0][:, 512:2048])
            nc.sync.dma_start(xs0[:, 2048:4096], x_v[0][:, 2048:4096])
            nc.sync.dma_start(xs0[:, 4096:8192], x_v[0][:, 4096:])

            # fire the lazy ACT_TABLE_LOAD (~1.3 us) during DMA warmup
            # instead of at the first real ReLU
            zz = work.tile([128, 1], F32, tag="zz")
            nc.vector.memset(zz[:], 0.0)
            warm = work.tile([128, 1], F16, tag="warm")
            nc.scalar.activation(warm[:], zz[:], relu, bias=zz[:])

            xs_t = [xs0] + [None] * (SLABS - 1)

            def load(s):
                xs_t[s] = slabs.tile(
                    [128, FREE], F16, tag="xs", name=f"xs{s}"
                )
                nc.gpsimd.dma_start(xs_t[s][:, : SLAB_COLS[s]], x_v[s])

            load(1)
            drain_i = 0
            for s in range(SLABS):
                # keep the load stream two slabs ahead of compute
                if s + 2 < SLABS:
                    load(s + 2)
                xs = xs_t[s]
                sbw = SLAB_SBW[s]
                ys = slabs.tile([128, FREE], F16, tag="ys")
                # stores at these superblock indices (end-exclusive col)
                nh = len(sbw) // 2
                if s == SLABS - 1:
                    marks = [3, 7, 10, 13]
                else:
                    marks = [nh - 1, len(sbw) - 1]
                st0 = 0
                col = 0
                # process in chunks of 4 superblocks: load each
                # stationary once per 4 matmuls instead of per matmul
                for c0 in range(0, len(sbw), 4):
                    chunk = sbw[c0 : c0 + 4]
                    ccol = col
                    h1s, ybs = [], []
                    for w in chunk:
                        h1_p = psum.tile([128, 512], F32, tag="h1")
                        nc.tensor.matmul(
                            h1_p[:, :w], bdw1[:], xs[:, ccol : ccol + w]
                        )
                        h1s.append(h1_p)
                        ccol += w
                    ccol = col
                    for j, w in enumerate(chunk):
                        yb = work.tile([128, 512], F16, tag="yb")
                        nc.scalar.activation(
                            yb[:, :w], h1s[j][:, :w], relu, bias=b1p[:]
                        )
                        ybs.append(yb)
                    h2s = []
                    for j, w in enumerate(chunk):
                        h2_p = psum.tile([128, 512], F32, tag="h2")
                        nc.tensor.matmul(h2_p[:, :w], bdwb[:], ybs[j][:, :w])
                        h2s.append(h2_p)
                    for j, w in enumerate(chunk):
                        # bias+cast drain: DVE, every 12th on ScalarE
                        if drain_i % 12 == 11:
                            nc.scalar.add(
                                ys[:, col : col + w], h2s[j][:, :w], b2p[:]
                            )
                        else:
                            nc.vector.tensor_scalar_add(
                                ys[:, col : col + w], h2s[j][:, :w], b2p[:]
                            )
                        drain_i += 1
                        col += w
                        if c0 + j in marks:
                            nc.sync.dma_start(
                                y_v[s][:, st0:col], ys[:, st0:col]
                            )
                            st0 = col

    _split_multi_waits(nc)
    return nc


_NC = None


def _get_program():
    global _NC
    if _NC is None:
        _NC = _build_program()
    return _NC


def _prepare_in_maps(inputs):
    feats = np.asarray(inputs["features"], dtype=np.float32)
    Wt = np.asarray(inputs["Wt"], dtype=np.float32)
    bt = np.asarray(inputs["bt"], dtype=np.float32)
    Wa = np.asarray(inputs["Wa"], dtype=np.float32)
    ba = np.asarray(inputs["ba"], dtype=np.float32)
    Wb = np.asarray(inputs["Wb"], dtype=np.float32)
    bb = np.asarray(inputs["bb"], dtype=np.float32)

    W1 = (Wa @ Wt).astype(np.float32)
    b1 = (Wa @ bt + ba).astype(np.float32)

    bdw1 = np.zeros((128, 128), np.float16)
    bdwb = np.zeros((128, 128), np.float16)
    for g in range(8):
        bdw1[16 * g : 16 * g + 16, 16 * g : 16 * g + 16] = W1.T.astype(np.float16)
        bdwb[16 * g : 16 * g + 16, 16 * g : 16 * g + 16] = Wb.T.astype(np.float16)
    b1p = np.tile(b1, 8).astype(np.float32).reshape(128, 1)
    b2p = np.tile(bb, 8).astype(np.float32).reshape(128, 1)

    # fp16 cast + channel-major-bundle layout, all cores at once:
    # x_cm[core][16g+c, t] = x[core][8t+g, c]  ==  x.reshape(T,128).T
    f16 = feats.astype(np.float16)
    shards = np.ascontiguousarray(
        f16.reshape(N_CORES, T_TOT, 128).transpose(0, 2, 1)
    ).reshape(N_CORES, 128 * T_TOT)
    return [
        {
            "x": shards[i],
            "bdw1": bdw1,
            "bdwb": bdwb,
            "b1p": b1p,
            "b2p": b2p,
        }
        for i in range(N_CORES)
    ]


def _run(inputs, trace=False):
    nc = _get_program()
    in_maps = _prepare_in_maps(inputs)
    res = run_bass_kernel_spmd(nc, in_maps, core_ids=list(range(N_CORES)), trace=trace)
    parts = [
        res.results[i]["y"]
        .reshape(128, T_TOT)
        .T.reshape(N_SHARD, C)
        .astype(np.float32)
        for i in range(N_CORES)
    ]
    out = np.concatenate(parts, axis=0)
    return out, res


def kernel(**inputs) -> np.ndarray:
    out, _ = _run(inputs, trace=False)
    return out


# revision 9
# speedup vs baseline: 1.2813x; 1.2813x over previous
"""Trainium2 Bass kernel for the dMaSIFConvBlock problem — fp16 I/O.

Effective math (points/nuv/ranges are dead inputs in the reference):
    h = features @ Wt.T + bt
    h = relu(h @ Wa.T + ba)
    out = h @ Wb.T + bb

Layers 1+2 fuse on the host into a single affine map (W1 = Wa@Wt,
b1 = Wa@bt + ba), so the device computes
    out = relu(features @ W1.T + b1) @ Wb.T + bb
a pointwise 16->16->16 MLP over 2M points.  Memory-bound; the rel-err
gate (2e-2) leaves ~100x precision headroom, so DRAM I/O is fp16:
8.1 MB in + 8.1 MB out per core at ~360 GB/s -> ~47 us/core floor
(vs ~94 us for f32 I/O).

The host also does the layout shuffle that the f32 baseline burned DVE
time on (InstStreamTranspose has no 2-byte fast path, so on-device
transposes would have become the new bottleneck at ~66 us):

  - Host casts features to fp16 and stores each core's 250,000-point
    shard channel-major-bundled: x_cm[16g+c, t] = x[8t+g, c], which
    is just x.reshape(T, 128).T.  Every DMA is a contiguous
    [128 partitions x T columns] slab; every matmul column holds 8
    points' 16-channel vectors on the 8 16-partition bundles.
  - The 16x16 weights are packed 8x along the diagonal of a 128x128
    fp16 stationary matrix; one N=512 fp16 matmul (1 col/cycle, same
    rate as f32r single-pass) applies a layer to 4096 points.
  - Layer-1 bias+ReLU runs on ScalarE ([128,1] f32 bias, fp16 out,
    which is also the rounding for the second matmul).
  - Layer-2 bias+drain (PSUM f32 -> SBUF fp16) runs on DVE
    tensor_scalar_add, with ~1 in 12 tiles peeled off to ScalarE so
    both engines stay near 38 us, under the ~47 us DMA budget.
    (GPSIMD has no PSUM port, so it cannot help drain.)
  - Output is stored channel-major as-is; the host undoes the layout
    with one strided copy and casts back to f32.

Loads ride the GPSIMD SWDGE ring and stores the sync-engine HWDGE
ring: two independent descriptor streams (a store waiting on compute
never stalls the load stream), and neither burns Act-sequencer time
on HWDGE descriptor generation (~0.7 us per DMA).  Loads are
whole-slab 2 MB transfers; stores are 1 MB half-slabs (quarters on
the last slab to shorten the tail).  The first slab is split finer
so the first matmul starts ~2 us in, and a dummy ReLU on a memset
tile fires the lazy ~1.3 us ACT_TABLE_LOAD during DMA warmup.
Matmuls run in chunks of 4 superblocks per stationary load (LDWEIGHTS
drops from 124 to 32 — at ~100 ns each it was ~20% of PE busy, and
PE at 95% busy was the v4 body bottleneck).  The point stream is cut
at exactly 250,000 points per core (61 full superblocks + one
18-column runt), no pad.

Environment quirk handled at build time: this walrus build rejects
instructions with more than one semaphore wait, while the Tile
scheduler freely attaches several; _split_multi_waits moves every
extra wait onto a standalone NoOp.
"""

import numpy as np

import concourse.bass as bass
import concourse.tile as tile
from concourse import mybir
from concourse.bass_utils import run_bass_kernel_spmd

N_TOTAL = 2_000_000
C = 16
N_CORES = 8
N_SHARD = N_TOTAL // N_CORES      # 250_000 points per core
T_TOT = N_SHARD // 8              # 31_250 columns per core, 8 pts/col
# 61 full 512-column superblocks + one 18-column runt; a slab is up
# to 16 superblocks = [128, 8192] fp16 = 2 MB.
SLAB_SBW = [[512] * 16 for _ in range(3)] + [[512] * 13 + [18]]
SLAB_COLS = [sum(w) for w in SLAB_SBW]        # 3x8192 + 6674
SLABS = len(SLAB_SBW)
FREE = 8192                                   # columns, full slab

F32 = mybir.dt.float32
F16 = mybir.dt.float16


def _split_multi_waits(nc):
    """Walrus here allows at most one semaphore wait per instruction.
    Move every extra wait onto its own NoOp placed just before the
    instruction on the same engine (waiting earlier on the same engine
    is equivalent: the waits' producers are other engines/queues)."""
    for func in nc.m.functions:
        for bb in func.blocks:
            out = []
            changed = False
            for inst in bb.instructions:
                si = inst.sync_info
                if si is not None and len(si.on_wait) > 1:
                    waits = list(si.on_wait)
                    for j, w in enumerate(waits[:-1]):
                        out.append(
                            mybir.InstNoOp(
                                name=f"{inst.name}-xw{j}",
                                sync_info=mybir.SyncInfo(on_wait=[w], on_update=[]),
                                bass_nofuse=True,
                                engine=inst.engine,
                            )
                        )
                    si.on_wait = [waits[-1]]
                    inst.sync_info = si
                    changed = True
                out.append(inst)
            if changed:
                bb.instructions = out


def _build_program():
    nc = bass.Bass()
    x_d = nc.dram_tensor("x", [128 * T_TOT], F16, kind="ExternalInput")
    y_d = nc.dram_tensor("y", [128 * T_TOT], F16, kind="ExternalOutput")
    w1_d = nc.dram_tensor("bdw1", [128, 128], F16, kind="ExternalInput")
    wb_d = nc.dram_tensor("bdwb", [128, 128], F16, kind="ExternalInput")
    b1_d = nc.dram_tensor("b1p", [128, 1], F32, kind="ExternalInput")
    b2_d = nc.dram_tensor("b2p", [128, 1], F32, kind="ExternalInput")

    x_2d = x_d.ap().rearrange("(p t) -> p t", p=128)
    y_2d = y_d.ap().rearrange("(p t) -> p t", p=128)
    # per-slab [128, cols] column windows (contiguous per-partition runs)
    x_v, y_v = [], []
    t0 = 0
    for cols in SLAB_COLS:
        x_v.append(x_2d[:, t0 : t0 + cols])
        y_v.append(y_2d[:, t0 : t0 + cols])
        t0 += cols
    relu = mybir.ActivationFunctionType.Relu

    with tile.TileContext(nc) as tc:
        with (
            tc.tile_pool(name="consts", bufs=1) as consts,
            tc.tile_pool(name="slabs", bufs=4) as slabs,
            tc.tile_pool(name="work", bufs=8) as work,
            tc.tile_pool(name="psum", bufs=4, space="PSUM") as psum,
        ):
            # cold start: slab-0 in pieces on the SWDGE ring, consts on
            # the sync ring, so the first matmul starts ~2 us in
            xs0 = slabs.tile([128, FREE], F16, tag="xs")
            nc.gpsimd.dma_start(xs0[:, :512], x_v[0][:, :512])
            bdw1 = consts.tile([128, 128], F16)
            nc.sync.dma_start(bdw1[:], w1_d.ap())
            b1p = consts.tile([128, 1], F32)
            nc.sync.dma_start(b1p[:], b1_d.ap())
            bdwb = consts.tile([128, 128], F16)
            nc.sync.dma_start(bdwb[:], wb_d.ap())
            b2p = consts.tile([128, 1], F32)
            nc.sync.dma_start(b2p[:], b2_d.ap())
            nc.gpsimd.dma_start(xs0[:, 512:2048], x_v[0][:, 512:2048])
            nc.gpsimd.dma_start(xs0[:, 2048:4096], x_v[0][:, 2048:4096])
            nc.gpsimd.dma_start(xs0[:, 4096:8192], x_v[0][:, 4096:])

            # fire the lazy ACT_TABLE_LOAD (~1.3 us) during DMA warmup
            # instead of at the first real ReLU
            zz = work.tile([128, 1], F32, tag="zz")
            nc.vector.memset(zz[:], 0.0)
            warm = work.tile([128, 1], F16, tag="warm")
            nc.scalar.activation(warm[:], zz[:], relu, bias=zz[:])

            xs_t = [xs0] + [None] * (SLABS - 1)

            def load(s):
                xs_t[s] = slabs.tile(
                    [128, FREE], F16, tag="xs", name=f"xs{s}"
                )
                nc.gpsimd.dma_start(xs_t[s][:, : SLAB_COLS[s]], x_v[s])

            load(1)
            drain_i = 0
            for s in range(SLABS):
                # keep the load stream two slabs ahead of compute
                if s + 2 < SLABS:
                    load(s + 2)
                xs = xs_t[s]
                sbw = SLAB_SBW[s]
                ys = slabs.tile([128, FREE], F16, tag="ys")
                # stores at these superblock indices (end-exclusive col)
                nh = len(sbw) // 2
                if s == SLABS - 1:
                    marks = [3, 7, 10, 13]
                else:
                    marks = [nh - 1, len(sbw) - 1]
                st0 = 0
                # software-pipelined PE stream with uniform lag 3:
                # mm2(i-3) is emitted right after mm1(i), so every mm2
                # has ~3 matmul times of slack for its activation to
                # land (the chunk-of-4 scheme gave the 4th mm2 zero)
                LAG = 3
                n_sb = len(sbw)
                cstart = [0] * n_sb
                c = 0
                for i, w in enumerate(sbw):
                    cstart[i] = c
                    c += w
                h1s = [None] * n_sb
                ybs = [None] * n_sb

                def stage1(i):
                    w = sbw[i]
                    h1s[i] = psum.tile([128, 512], F32, tag="h1", name=f"h1_{s}_{i}")
                    nc.tensor.matmul(
                        h1s[i][:, :w], bdw1[:], xs[:, cstart[i] : cstart[i] + w]
                    )
                    ybs[i] = work.tile([128, 512], F16, tag="yb", name=f"yb_{s}_{i}")
                    nc.scalar.activation(
                        ybs[i][:, :w], h1s[i][:, :w], relu, bias=b1p[:]
                    )

                def stage2(j):
                    nonlocal st0, drain_i
                    w = sbw[j]
                    h2_p = psum.tile([128, 512], F32, tag="h2", name=f"h2_{s}_{j}")
                    nc.tensor.matmul(h2_p[:, :w], bdwb[:], ybs[j][:, :w])
                    # bias+cast drain: DVE, every 12th on ScalarE
                    if drain_i % 12 == 11:
                        nc.scalar.add(
                            ys[:, cstart[j] : cstart[j] + w], h2_p[:, :w], b2p[:]
                        )
                    else:
                        nc.vector.tensor_scalar_add(
                            ys[:, cstart[j] : cstart[j] + w], h2_p[:, :w], b2p[:]
                        )
                    drain_i += 1
                    if j in marks:
                        end = cstart[j] + w
                        nc.sync.dma_start(y_v[s][:, st0:end], ys[:, st0:end])
                        st0 = end

                for i in range(n_sb + LAG):
                    if i < n_sb:
                        stage1(i)
                    if i >= LAG:
                        stage2(i - LAG)

    _split_multi_waits(nc)
    return nc


_NC = None


def _get_program():
    global _NC
    if _NC is None:
        _NC = _build_program()
    return _NC


def _prepare_in_maps(inputs):
    feats = np.asarray(inputs["features"], dtype=np.float32)
    Wt = np.asarray(inputs["Wt"], dtype=np.float32)
    bt = np.asarray(inputs["bt"], dtype=np.float32)
    Wa = np.asarray(inputs["Wa"], dtype=np.float32)
    ba = np.asarray(inputs["ba"], dtype=np.float32)
    Wb = np.asarray(inputs["Wb"], dtype=np.float32)
    bb = np.asarray(inputs["bb"], dtype=np.float32)

    W1 = (Wa @ Wt).astype(np.float32)
    b1 = (Wa @ bt + ba).astype(np.float32)

    bdw1 = np.zeros((128, 128), np.float16)
    bdwb = np.zeros((128, 128), np.float16)
    for g in range(8):
        bdw1[16 * g : 16 * g + 16, 16 * g : 16 * g + 16] = W1.T.astype(np.float16)
        bdwb[16 * g : 16 * g + 16, 16 * g : 16 * g + 16] = Wb.T.astype(np.float16)
    b1p = np.tile(b1, 8).astype(np.float32).reshape(128, 1)
    b2p = np.tile(bb, 8).astype(np.float32).reshape(128, 1)

    # fp16 cast + channel-major-bundle layout, all cores at once:
    # x_cm[core][16g+c, t] = x[core][8t+g, c]  ==  x.reshape(T,128).T
    f16 = feats.astype(np.float16)
    shards = np.ascontiguousarray(
        f16.reshape(N_CORES, T_TOT, 128).transpose(0, 2, 1)
    ).reshape(N_CORES, 128 * T_TOT)
    return [
        {
            "x": shards[i],
            "bdw1": bdw1,
            "bdwb": bdwb,
            "b1p": b1p,
            "b2p": b2p,
        }
        for i in range(N_CORES)
    ]


def _run(inputs, trace=False):
    nc = _get_program()
    in_maps = _prepare_in_maps(inputs)
    res = run_bass_kernel_spmd(nc, in_maps, core_ids=list(range(N_CORES)), trace=trace)
    parts = [
        res.results[i]["y"]
        .reshape(128, T_TOT)
        .T.reshape(N_SHARD, C)
        .astype(np.float32)
        for i in range(N_CORES)
    ]
    out = np.concatenate(parts, axis=0)
    return out, res


def kernel(**inputs) -> np.ndarray:
    out, _ = _run(inputs, trace=False)
    return out
